# revision 2
# baseline (speedup 1.0000x reference)
"""Debayer 3x3 kernel for Trainium2 (Bass/Tile), batch-sharded over 8 NeuronCores.

Reference semantics: 1->5 channel 3x3 conv (identity, plus-4, diag-4,
horiz-2, vert-2) over an edge-padded Bayer frame, then per-2x2-parity
channel select into RGB.

v2 strategy (memory-regime; cuts HBM bytes 16 -> 6 per pixel vs v1):
  * Identity channel never touches the device. Per pixel exactly one of
    R,G,B equals x (R at (e,e), G at (e,o)/(o,e), B at (o,o)); the host
    fills those from the original f32 input. The device ships only the
    two non-trivial values per pixel, as two planes:
      A  = non-identity R-or-G:  (e,e)=c1  (e,o)=c3  (o,e)=c4  (o,o)=c2
      B2 = non-identity B-or-G:  (e,e)=c2  (e,o)=c4  (o,e)=c3  (o,o)=c1
  * fp16 I/O end to end (correctness gate 2e-2; fp16 costs ~4e-4).
  * Host pre-scales x by 1/4, so with q = x/4:
      SQ[r,j] = q[r,j-1]+q[r,j+1]   (horiz/4)   VQ = vert/4
      c1 = SQ+VQ   c2 = SQ[up]+SQ[down]   c3 = 2*SQ   c4 = 2*VQ
    -> device does only DVE adds plus four Act x2 muls per slice.
  * Host column-deinterleaves each patch (even cols, then odd cols) so
    every engine AP is stride-1 in the last dim: the DVE 2x 16-bit mode
    needs packed operands.

Device layout: each padded 1090x1922 image is tiled into 128 partitions
x 4 col-slices of (36 rows x 122 cols) patches:
  partition p = 32*q + b  (col-quarter q in 0..3, row-band b in 0..31)
  band b   -> image rows [34b, 34b+34)        (patch has +-1 halo rows)
  slice s  -> image cols [480q+120s, +120)    (patch has +-1 halo cols)
Patch cols are stored deinterleaved: [36, 2, 61] with [:,0,:] = patch
cols 0,2,..,120 and [:,1,:] = 1,3,..,121. Out col j=2v+p corresponds to
patch col j+1, so horizontal neighbor sums stay parity-pure and packed.
"""

import numpy as np

H, W = 1088, 1920
NB = 32          # row bands per column-quarter
BH = 34          # output rows per band
NQ = 4           # column quarters
NS = 4           # col slices per patch
SW = 120         # output cols per slice
HSW = SW // 2    # 60 per col-parity
PR = BH + 2      # patch rows (with halo)
HPC = 61         # patch cols per parity (122 total, with halo)

_NC_CACHE = {}
LAST_RESULTS = None


def _build(reps=1, *, in_bufs=3, mid_bufs=2, out_bufs=3, **_ignored):
    """Build the Bass module. reps>1 repeats the whole pipeline (bench only:
    amortizes per-dispatch overhead out of wall-clock measurements)."""
    key = (reps, in_bufs, mid_bufs, out_bufs)
    if key in _NC_CACHE:
        return _NC_CACHE[key]
    import concourse.bacc as bacc
    import concourse.mybir as mybir
    import concourse.tile as tile
    from concourse._compat import get_trn_type

    f16 = mybir.dt.float16
    nc = bacc.Bacc(get_trn_type() or "TRN2", target_bir_lowering=False, debug=False)
    xin = nc.dram_tensor("xprep", [128, NS, PR, 2, HPC], f16, kind="ExternalInput")
    yout = nc.dram_tensor("yout", [128, NS, 2, BH, 2, HSW], f16, kind="ExternalOutput")
    # bench-only: earlier reps dump to internal scratch so no two reps write
    # the same DRAM (WAW races hang the exec unit)
    ydumps = [
        nc.dram_tensor(f"ydump{r}", [128, NS, 2, BH, 2, HSW], f16, kind="Internal")
        for r in range(reps - 1)
    ]

    # out-row parity slices (within BH output rows)
    E_, O_ = slice(0, BH, 2), slice(1, BH, 2)
    # patch-row slice for out rows of given parity (out row i -> patch row i+1)
    pE, pO = slice(1, PR - 1, 2), slice(2, PR, 2)
    # SQ rows for diag channel: out row i needs patch rows i and i+2
    dE0, dE1 = slice(0, PR - 2, 2), slice(2, PR, 2)      # even out rows
    dO0, dO1 = slice(1, PR - 1, 2), slice(3, PR, 2)      # odd out rows

    with tile.TileContext(nc) as tc:
        with tc.tile_pool(name="pin", bufs=in_bufs) as pin, \
             tc.tile_pool(name="pmid", bufs=mid_bufs) as pmid, \
             tc.tile_pool(name="pout", bufs=out_bufs) as pout:

            def load(j):
                t = pin.tile([128, PR, 2, HPC], f16, tag="inp", name=f"inp{j}")
                nc.sync.dma_start(out=t[:], in_=xin[:, j % NS])
                return t

            cur = load(0)
            for j in range(NS * reps):
                k = j % NS
                r = j // NS
                ytgt = yout if r == reps - 1 else ydumps[r]
                nxt = load(j + 1) if j + 1 < NS * reps else None
                Q = cur  # [128, PR, 2, HPC], q = x/4, col-deinterleaved
                # SQ[p,r,c,v] = q horiz-pair sum at patch row r, col par c
                SQ = pmid.tile([128, PR, 2, HSW], f16, tag="sq", name=f"sq{k}")
                nc.vector.tensor_add(SQ[:], Q[:, :, :, 0:HSW], Q[:, :, :, 1:HSW + 1])
                # VQ[p,i,c,v] = q vert-pair sum at out row i, col parity c
                # (out col 2v -> patch col 2v+1 = odd par; 2v+1 -> even par v+1)
                VQ = pmid.tile([128, BH, 2, HSW], f16, tag="vq", name=f"vq{k}")
                nc.vector.tensor_add(VQ[:, :, 0, :], Q[:, 0:BH, 1, 0:HSW],
                                     Q[:, 2:PR, 1, 0:HSW])
                nc.vector.tensor_add(VQ[:, :, 1, :], Q[:, 0:BH, 0, 1:HSW + 1],
                                     Q[:, 2:PR, 0, 1:HSW + 1])

                Y = pout.tile([128, 2, BH, 2, HSW], f16, tag="y", name=f"y{k}")
                # Act engine: c3/c4 = 2*SQ / 2*VQ quarters (depend only on SQ/VQ)
                nc.scalar.mul(Y[:, 0, E_, 1, :], SQ[:, pE, 1, :], 2.0)   # A(E,o)=c3
                nc.scalar.mul(Y[:, 0, O_, 0, :], VQ[:, O_, 0, :], 2.0)   # A(O,e)=c4
                nc.scalar.mul(Y[:, 1, O_, 0, :], SQ[:, pO, 0, :], 2.0)   # B(O,e)=c3
                nc.scalar.mul(Y[:, 1, E_, 1, :], VQ[:, E_, 1, :], 2.0)   # B(E,o)=c4
                # DVE: c1/c2 quarter adds
                nc.vector.tensor_add(Y[:, 0, E_, 0, :], SQ[:, pE, 0, :],
                                     VQ[:, E_, 0, :])                    # A(E,e)=c1
                nc.vector.tensor_add(Y[:, 0, O_, 1, :], SQ[:, dO0, 1, :],
                                     SQ[:, dO1, 1, :])                   # A(O,o)=c2
                nc.vector.tensor_add(Y[:, 1, E_, 0, :], SQ[:, dE0, 0, :],
                                     SQ[:, dE1, 0, :])                   # B(E,e)=c2
                nc.vector.tensor_add(Y[:, 1, O_, 1, :], SQ[:, pO, 1, :],
                                     VQ[:, O_, 1, :])                    # B(O,o)=c1
                nc.sync.dma_start(out=ytgt[:, k], in_=Y[:])

                cur = nxt

    nc.compile()
    _NC_CACHE[key] = nc
    return nc


def _prep_inputs(x):
    """(B,1,1088,1920) f32 -> (B,128,NS,PR,2,HPC) fp16 patches of x/4,
    edge padded, column-deinterleaved."""
    Bn = x.shape[0]
    q = (x[:, 0] * np.float32(0.25)).astype(np.float16)
    qpad = np.pad(q, ((0, 0), (1, 1), (1, 1)), mode="edge")  # (B,1090,1922)
    xprep = np.empty((Bn, 128, NS, PR, 2, HPC), np.float16)
    st = qpad.strides
    for qi in range(NQ):
        for s in range(NS):
            c0 = 480 * qi + SW * s
            block = qpad[:, :, c0:c0 + 2 * HPC]
            v = np.lib.stride_tricks.as_strided(
                block, shape=(Bn, NB, PR, 2 * HPC),
                strides=(st[0], BH * st[1], st[1], st[2]))
            xprep[:, qi * NB:(qi + 1) * NB, s] = (
                v.reshape(Bn, NB, PR, HPC, 2).transpose(0, 1, 2, 4, 3))
    return xprep


def _assemble(y, x):
    """y (128,NS,2,BH,2,HSW) fp16 device planes + x (1088,1920) f32 original
    -> (3,1088,1920) f32 RGB."""
    AB = np.empty((2, H, W), np.float32)
    for qi in range(NQ):
        blk = y[qi * NB:(qi + 1) * NB]               # (NB,NS,2,BH,2,HSW)
        for ch in range(2):
            part = blk[:, :, ch]                     # (NB,NS,BH,2,HSW)
            AB[ch, :, 480 * qi:480 * (qi + 1)] = (
                part.transpose(0, 2, 1, 4, 3).reshape(H, 480))
    A, B2 = AB[0], AB[1]
    out = np.empty((3, H, W), np.float32)
    # R: identity at (e,e), else A
    out[0] = A
    out[0][0::2, 0::2] = x[0::2, 0::2]
    # G: identity at (e,o)/(o,e); c1 from A at (e,e), from B2 at (o,o)
    out[1][0::2, 1::2] = x[0::2, 1::2]
    out[1][1::2, 0::2] = x[1::2, 0::2]
    out[1][0::2, 0::2] = A[0::2, 0::2]
    out[1][1::2, 1::2] = B2[1::2, 1::2]
    # B: identity at (o,o), else B2
    out[2] = B2
    out[2][1::2, 1::2] = x[1::2, 1::2]
    return out


def kernel(x, kernels=None, index=None, **_unused):
    global LAST_RESULTS
    x = np.ascontiguousarray(np.asarray(x), dtype=np.float32)
    Bn = x.shape[0]
    xprep = _prep_inputs(x)
    nc = _build()
    from concourse.bass_utils import run_bass_kernel_spmd
    in_maps = [{"xprep": xprep[i]} for i in range(Bn)]
    res = run_bass_kernel_spmd(nc, in_maps, core_ids=list(range(Bn)))
    LAST_RESULTS = res
    out = np.empty((Bn, 3, H, W), np.float32)
    for i in range(Bn):
        out[i] = _assemble(res.results[i]["yout"], x[i, 0])
    return out


# revision 7
# speedup vs baseline: 2.1839x; 2.1839x over previous
"""Debayer 3x3 kernel for Trainium2 (Bass/Tile), batch-sharded over 8 NeuronCores.

Reference semantics: 1->5 channel 3x3 conv (identity, plus-4, diag-4,
horiz-2, vert-2) over an edge-padded Bayer frame, then per-2x2-parity
channel select into RGB.

v2 strategy (memory-regime; cuts HBM bytes 16 -> 6 per pixel vs v1):
  * Identity channel never touches the device. Per pixel exactly one of
    R,G,B equals x (R at (e,e), G at (e,o)/(o,e), B at (o,o)); the host
    fills those from the original f32 input. The device ships only the
    two non-trivial values per pixel, as two planes:
      A  = non-identity R-or-G:  (e,e)=c1  (e,o)=c3  (o,e)=c4  (o,o)=c2
      B2 = non-identity B-or-G:  (e,e)=c2  (e,o)=c4  (o,e)=c3  (o,o)=c1
  * fp16 I/O end to end (correctness gate 2e-2; fp16 costs ~4e-4).
  * Host pre-scales x by 1/4, so with q = x/4:
      SQ[r,j] = q[r,j-1]+q[r,j+1]   (horiz/4)   VQ = vert/4
      c1 = SQ+VQ   c2 = SQ[up]+SQ[down]   c3 = 2*SQ   c4 = 2*VQ
    -> device does only DVE adds plus four Act x2 muls per slice.
  * Host column-deinterleaves each patch (even cols, then odd cols) so
    every engine AP is stride-1 in the last dim: the DVE 2x 16-bit mode
    needs packed operands.

Device layout: each padded 1090x1922 image is tiled into 128 partitions
x 4 col-slices of (36 rows x 122 cols) patches:
  partition p = 32*q + b  (col-quarter q in 0..3, row-band b in 0..31)
  band b   -> image rows [34b, 34b+34)        (patch has +-1 halo rows)
  slice s  -> image cols [480q+120s, +120)    (patch has +-1 halo cols)
Patch cols are stored deinterleaved: [36, 2, 61] with [:,0,:] = patch
cols 0,2,..,120 and [:,1,:] = 1,3,..,121. Out col j=2v+p corresponds to
patch col j+1, so horizontal neighbor sums stay parity-pure and packed.
"""

import numpy as np

H, W = 1088, 1920
NB = 32          # row bands per column-quarter
BH = 34          # output rows per band
NQ = 4           # column quarters
NS = 4           # col slices per patch
SW = 120         # output cols per slice
HSW = SW // 2    # 60 per col-parity
PR = BH + 2      # patch rows (with halo)
HPC = 61         # patch cols per parity (122 total, with halo)

_NC_CACHE = {}
LAST_RESULTS = None


def _build(reps=1, *, in_bufs=3, mid_bufs=2, out_bufs=3,
           no_compute=False, out_small=False, out_u8=False, **_ignored):
    """Build the Bass module. reps>1 repeats the whole pipeline (bench only:
    amortizes per-dispatch overhead out of wall-clock measurements).
    Probe flags (bench only): no_compute = DMA skeleton; out_small = full
    compute but 1-row output DMA; out_u8 = uint8 output tensor."""
    key = (reps, in_bufs, mid_bufs, out_bufs, no_compute, out_small, out_u8)
    if key in _NC_CACHE:
        return _NC_CACHE[key]
    import concourse.bacc as bacc
    import concourse.mybir as mybir
    import concourse.tile as tile
    from concourse._compat import get_trn_type

    f16 = mybir.dt.float16
    odt = mybir.dt.uint8 if out_u8 else f16
    nc = bacc.Bacc(get_trn_type() or "TRN2", target_bir_lowering=False, debug=False)
    xin = nc.dram_tensor("xprep", [128, NS, PR, 2, HPC], f16, kind="ExternalInput")
    yout = nc.dram_tensor("yout", [128, NS, 2, BH, 2, HSW], odt, kind="ExternalOutput")
    # bench-only: earlier reps dump to internal scratch so no two reps write
    # the same DRAM (WAW races hang the exec unit)
    ydumps = [
        nc.dram_tensor(f"ydump{r}", [128, NS, 2, BH, 2, HSW], odt, kind="Internal")
        for r in range(reps - 1)
    ]

    # out-row parity slices (within BH output rows)
    E_, O_ = slice(0, BH, 2), slice(1, BH, 2)
    # patch-row slice for out rows of given parity (out row i -> patch row i+1)
    pE, pO = slice(1, PR - 1, 2), slice(2, PR, 2)
    # SQ rows for diag channel: out row i needs patch rows i and i+2
    dE0, dE1 = slice(0, PR - 2, 2), slice(2, PR, 2)      # even out rows
    dO0, dO1 = slice(1, PR - 1, 2), slice(3, PR, 2)      # odd out rows

    with tile.TileContext(nc) as tc:
        with tc.tile_pool(name="pin", bufs=in_bufs) as pin, \
             tc.tile_pool(name="pmid", bufs=mid_bufs) as pmid, \
             tc.tile_pool(name="pout", bufs=out_bufs) as pout:

            def load(j):
                t = pin.tile([128, PR, 2, HPC], f16, tag="inp", name=f"inp{j}")
                nc.sync.dma_start(out=t[:], in_=xin[:, j % NS])
                return t

            cur = load(0)
            for j in range(NS * reps):
                k = j % NS
                r = j // NS
                ytgt = yout if r == reps - 1 else ydumps[r]
                nxt = load(j + 1) if j + 1 < NS * reps else None
                Q = cur  # [128, PR, 2, HPC], q = x/4, col-deinterleaved
                if no_compute:
                    # bench-only: DMA skeleton (touch input once so it's live)
                    Yd = pout.tile([128, 2, BH, 2, HSW], odt, tag="y", name=f"y{k}")
                    nc.scalar.mul(Yd[:, 0, 0:1, 0, :], Q[:, 0:1, 0, 0:HSW], 1.0)
                    nc.sync.dma_start(out=ytgt[:, k], in_=Yd[:])
                    cur = nxt
                    continue
                # SQ[p,r,c,v] = q horiz-pair sum at patch row r, col par c
                SQ = pmid.tile([128, PR, 2, HSW], f16, tag="sq", name=f"sq{k}")
                nc.vector.tensor_add(SQ[:], Q[:, :, :, 0:HSW], Q[:, :, :, 1:HSW + 1])
                # VQ[p,i,c,v] = q vert-pair sum at out row i, col parity c
                # (out col 2v -> patch col 2v+1 = odd par; 2v+1 -> even par v+1)
                VQ = pmid.tile([128, BH, 2, HSW], f16, tag="vq", name=f"vq{k}")
                nc.vector.tensor_add(VQ[:, :, 0, :], Q[:, 0:BH, 1, 0:HSW],
                                     Q[:, 2:PR, 1, 0:HSW])
                nc.vector.tensor_add(VQ[:, :, 1, :], Q[:, 0:BH, 0, 1:HSW + 1],
                                     Q[:, 2:PR, 0, 1:HSW + 1])

                Y = pout.tile([128, 2, BH, 2, HSW], f16, tag="y", name=f"y{k}")
                # Act engine: c3/c4 = 2*SQ / 2*VQ quarters (depend only on SQ/VQ)
                nc.scalar.mul(Y[:, 0, E_, 1, :], SQ[:, pE, 1, :], 2.0)   # A(E,o)=c3
                nc.scalar.mul(Y[:, 0, O_, 0, :], VQ[:, O_, 0, :], 2.0)   # A(O,e)=c4
                nc.scalar.mul(Y[:, 1, O_, 0, :], SQ[:, pO, 0, :], 2.0)   # B(O,e)=c3
                nc.scalar.mul(Y[:, 1, E_, 1, :], VQ[:, E_, 1, :], 2.0)   # B(E,o)=c4
                # DVE: c1/c2 quarter adds
                nc.vector.tensor_add(Y[:, 0, E_, 0, :], SQ[:, pE, 0, :],
                                     VQ[:, E_, 0, :])                    # A(E,e)=c1
                nc.vector.tensor_add(Y[:, 0, O_, 1, :], SQ[:, dO0, 1, :],
                                     SQ[:, dO1, 1, :])                   # A(O,o)=c2
                nc.vector.tensor_add(Y[:, 1, E_, 0, :], SQ[:, dE0, 0, :],
                                     SQ[:, dE1, 0, :])                   # B(E,e)=c2
                nc.vector.tensor_add(Y[:, 1, O_, 1, :], SQ[:, pO, 1, :],
                                     VQ[:, O_, 1, :])                    # B(O,o)=c1
                if out_small:
                    nc.sync.dma_start(out=ytgt[:, k, :, 0:1], in_=Y[:, :, 0:1])
                else:
                    nc.sync.dma_start(out=ytgt[:, k], in_=Y[:])

                cur = nxt

    nc.compile()
    _NC_CACHE[key] = nc
    return nc


def _prep_inputs(x):
    """(B,1,1088,1920) f32 -> (B,128,NS,PR,2,HPC) fp16 patches of x/4,
    edge padded, column-deinterleaved."""
    Bn = x.shape[0]
    q = (x[:, 0] * np.float32(0.25)).astype(np.float16)
    qpad = np.pad(q, ((0, 0), (1, 1), (1, 1)), mode="edge")  # (B,1090,1922)
    xprep = np.empty((Bn, 128, NS, PR, 2, HPC), np.float16)
    st = qpad.strides
    for qi in range(NQ):
        for s in range(NS):
            c0 = 480 * qi + SW * s
            block = qpad[:, :, c0:c0 + 2 * HPC]
            v = np.lib.stride_tricks.as_strided(
                block, shape=(Bn, NB, PR, 2 * HPC),
                strides=(st[0], BH * st[1], st[1], st[2]))
            xprep[:, qi * NB:(qi + 1) * NB, s] = (
                v.reshape(Bn, NB, PR, HPC, 2).transpose(0, 1, 2, 4, 3))
    return xprep


def _assemble(y, x):
    """y (128,NS,2,BH,2,HSW) fp16 device planes + x (1088,1920) f32 original
    -> (3,1088,1920) f32 RGB."""
    AB = np.empty((2, H, W), np.float32)
    for qi in range(NQ):
        blk = y[qi * NB:(qi + 1) * NB]               # (NB,NS,2,BH,2,HSW)
        for ch in range(2):
            part = blk[:, :, ch]                     # (NB,NS,BH,2,HSW)
            AB[ch, :, 480 * qi:480 * (qi + 1)] = (
                part.transpose(0, 2, 1, 4, 3).reshape(H, 480))
    A, B2 = AB[0], AB[1]
    out = np.empty((3, H, W), np.float32)
    # R: identity at (e,e), else A
    out[0] = A
    out[0][0::2, 0::2] = x[0::2, 0::2]
    # G: identity at (e,o)/(o,e); c1 from A at (e,e), from B2 at (o,o)
    out[1][0::2, 1::2] = x[0::2, 1::2]
    out[1][1::2, 0::2] = x[1::2, 0::2]
    out[1][0::2, 0::2] = A[0::2, 0::2]
    out[1][1::2, 1::2] = B2[1::2, 1::2]
    # B: identity at (o,o), else B2
    out[2] = B2
    out[2][1::2, 1::2] = x[1::2, 1::2]
    return out


def kernel(x, kernels=None, index=None, **_unused):
    global LAST_RESULTS
    x = np.ascontiguousarray(np.asarray(x), dtype=np.float32)
    Bn = x.shape[0]
    xprep = _prep_inputs(x)
    nc = _build()
    from concourse.bass_utils import run_bass_kernel_spmd
    in_maps = [{"xprep": xprep[i]} for i in range(Bn)]
    res = run_bass_kernel_spmd(nc, in_maps, core_ids=list(range(Bn)))
    LAST_RESULTS = res
    out = np.empty((Bn, 3, H, W), np.float32)
    for i in range(Bn):
        out[i] = _assemble(res.results[i]["yout"], x[i, 0])
    return out
